# revision 18
# baseline (speedup 1.0000x reference)
"""Tacotron2-style decoder on 8 TRN2 NeuronCores (self-contained).

Strategy: model-parallel over LSTM gate/hidden dims (8-way, interleaved so core c
owns h-dims [128c:128c+128] of both LSTMs), batch-sharded attention (2 rows per
core), bf16 weights resident in SBUF.  Teacher forcing lets the prenet and
prenet@wih run as one batched pass before the scan; the projection runs batched
after it.  Per step: AllGather A = {pre_h_t^T, post_h_{t-1}^T} shards,
AllGather B = ctx_t^T.  Dropout masks are precomputed on host with JAX threefry
(bit-exact vs the reference).
"""
import os
import numpy as np
import ml_dtypes

import concourse.bass as bass
import concourse.tile as tile
import concourse.mybir as mybir
from concourse import bacc
from concourse.bass import ds
from concourse.bass_utils import run_bass_kernel_spmd

bf16 = mybir.dt.bfloat16
f32 = mybir.dt.float32
AF = mybir.ActivationFunctionType
ALU = mybir.AluOpType
BF = ml_dtypes.bfloat16

NC = 8
B = 16
FEAT = 1025
ENC_D = 512
ENC_T = 256
PRENET = 256
H = 1024          # both LSTM hidden sizes
HSH = H // NC     # 128: per-core hidden shard
GSH = 4 * HSH     # 512: per-core gate shard (i|f|g|o chunks of 128)
ATTN = 128
PROJ_W = 129      # per-core projection width (col start = 128c; overlaps ok)
XK = 1152         # padded prenet input K (1025 -> 9*128)

_nc_cache = {}


def _bf(x):
    return np.ascontiguousarray(np.asarray(x, np.float32).astype(BF))


def _ktile(w):
    """[K, N] -> [128, K//128, N] (partition-major within each K tile)."""
    K, N = w.shape
    assert K % 128 == 0
    return np.ascontiguousarray(w.reshape(K // 128, 128, N).transpose(1, 0, 2))


def build_nc(n_steps):
    T = n_steps
    T_loop = int(os.environ.get("KLOOPT", str(T)))
    M2 = 16 * T                     # prenet rows, (t, b) with b inner
    nc = bacc.Bacc("TRN2", target_bir_lowering=False, debug=False, num_devices=NC)

    def inp(name, shape, dtype):
        return nc.dram_tensor(name, list(shape), dtype, kind="ExternalInput")

    xT = inp("xT", [128, 9, M2], bf16)
    w0 = inp("w0", [128, 9, PRENET], bf16)
    w1 = inp("w1", [128, 2, PRENET], bf16)
    m1T = inp("m1T", [128, 2, M2], bf16)
    m2T = inp("m2T", [128, 2, M2], bf16)
    wih_pre_x = inp("wih_pre_x", [128, 2, GSH], bf16)
    preb_row = inp("preb_row", [128, GSH], bf16)
    wih_pre_ctx = inp("wih_pre_ctx", [128, 4, GSH], bf16)
    whh_pre = inp("whh_pre", [128, 8, GSH], bf16)
    wih_post_h = inp("wih_post_h", [128, 8, GSH], bf16)
    wih_post_ctx = inp("wih_post_ctx", [128, 4, GSH], bf16)
    whh_post = inp("whh_post", [128, 8, GSH], bf16)
    postb_row = inp("postb_row", [128, GSH], bf16)
    wq = inp("wq", [128, 8, ATTN], bf16)
    wmem = inp("wmem", [128, 4, ATTN], bf16)
    wf = inp("wf", [128, ATTN], bf16)
    vattn = inp("vattn", [128, 1], bf16)
    mem_t = inp("mem_t", [128, 4, ENC_D], bf16)
    enc_dT = inp("enc_dT", [128, 4, 512], bf16)
    lenmask = inp("lenmask", [128, 512], bf16)
    lenmaskT = inp("lenmaskT", [128, 4], bf16)
    hmask = inp("hmask", [T, 16, 256], f32)
    ident = inp("ident", [128, 128], bf16)
    ones_row = inp("ones_row", [128, 128], bf16)
    ones_f32 = inp("ones_f32", [128, 128], f32)
    wproj = inp("wproj", [128, 12, PROJ_W], bf16)
    projb_row = inp("projb_row", [128, PROJ_W], bf16)

    out_proj = nc.dram_tensor("out_proj", [M2, PROJ_W], f32, kind="ExternalOutput")
    out_align = nc.dram_tensor("out_align", [T, 2, ENC_T], f32, kind="ExternalOutput")

    RG = [list(range(NC))]

    with tile.TileContext(nc) as tc:
        with tc.tile_pool(name="wsb", bufs=1) as wsb, \
             tc.tile_pool(name="pers", bufs=1) as pers, \
             tc.tile_pool(name="rot", bufs=4) as rot, \
             tc.tile_pool(name="rotp", bufs=2) as rotp, \
             tc.tile_pool(name="psA", bufs=2, space="PSUM") as psA, \
             tc.tile_pool(name="psB", bufs=3, space="PSUM") as psB, \
             tc.tile_pool(name="psT", bufs=2, space="PSUM") as psT, \
             tc.tile_pool(name="psT2", bufs=1, space="PSUM") as psT2, \
             tc.tile_pool(name="dram", bufs=2, space="DRAM") as dram, \
             tc.tile_pool(name="dram1", bufs=1, space="DRAM") as dram1:

            def load(t, shape, dtype=bf16):
                s = wsb.tile(list(shape), dtype, name=f"L_{t.name}")
                nc.sync.dma_start(s[:], t[:])
                return s

            w0_s = load(w0, [128, 9, PRENET])
            w1_s = load(w1, [128, 2, PRENET])
            wpx_s = load(wih_pre_x, [128, 2, GSH])
            prb_s = load(preb_row, [128, GSH])
            wpc_s = load(wih_pre_ctx, [128, 4, GSH])
            whp_s = load(whh_pre, [128, 8, GSH])
            wph_s = load(wih_post_h, [128, 8, GSH])
            wpc2_s = load(wih_post_ctx, [128, 4, GSH])
            whq_s = load(whh_post, [128, 8, GSH])
            pob_s = load(postb_row, [128, GSH])
            wq_s = load(wq, [128, 8, ATTN])
            wmem_s = load(wmem, [128, 4, ATTN])
            wf_s = load(wf, [128, ATTN])
            v_s = load(vattn, [128, 1])
            mem_s = load(mem_t, [128, 4, ENC_D])
            encd_s = load(enc_dT, [128, 4, 512])
            lm_s = load(lenmask, [128, 512])
            lmT_s = load(lenmaskT, [128, 4])
            id_s = load(ident, [128, 128])
            ones_s = load(ones_row, [128, 128])
            onesf_s = load(ones_f32, [128, 128], f32)
            wproj_s = load(wproj, [128, 12, PROJ_W])
            pjb_s = load(projb_row, [128, PROJ_W])

            known_d = dram1.tile([M2, GSH], bf16, name="known_d")
            histA_d = nc.dram_tensor("histA_d", [T + 1, 1024, 32], bf16,
                                     kind="Internal", addr_space="Shared")[:]
            histC_d = nc.dram_tensor("histC_d", [T + 1, 1024, 8], bf16,
                                     kind="Internal", addr_space="Shared")[:]
            locrep_d = [dram1.tile([2, 31, 2, 286], bf16, name=f"locrep{i}") for i in range(2)]

            def zt(name, shape, dtype=f32, pool=pers):
                s = pool.tile(list(shape), dtype, name=name)
                nc.any.memset(s[:], 0.0)
                return s

            pre_c = zt("pre_c", [16, HSH])
            post_c = zt("post_c", [16, HSH])
            cum_f = zt("cum_f", [1, 2, ENC_T])
            align_bf = [zt(f"align_bf{b}", [1, 286], bf16) for b in range(2)]
            cum_bf = [zt(f"cum_bf{b}", [1, 286], bf16) for b in range(2)]
            exchA_src = [zt(f"exchA{i}", [128, 32], bf16) for i in range(2)]
            exchB_src = [zt(f"exchB{i}", [128, 8], bf16) for i in range(2)]
            known_buf = [zt(f"knownb{i}", [128, GSH], bf16) for i in range(3)]
            imc = [zt(f"imc{i}", [128, 512], bf16) for i in range(2)]
            sums_pad = [zt(f"sums{i}", [128, 2]) for i in range(2)]
            curA0 = zt("curA0", [128, 8, 32], bf16)
            ctxT0 = zt("ctxT0", [128, 4, 16], bf16)
            zero8 = zt("zero8", [128, 64], bf16)

            # align0 / cum0: weight 1.0 at enc position 0 (offset 15 in padded buf)
            nc.any.memset(cum_f[0:1, :, 0:1], 1.0)
            for b in range(2):
                nc.any.memset(align_bf[b][0:1, 15:16], 1.0)
                nc.any.memset(cum_bf[b][0:1, 15:16], 1.0)

            nc.sync.dma_start(
                bass.AP(histC_d.tensor, histC_d.offset, [[64, 128], [1, 64]]),
                zero8[:])

            pid = nc.partition_id()
            boff = pid * 2

            # ================= PREFIX =================
            NB = 400 if M2 % 400 == 0 else M2
            nblocks = M2 // NB
            out2T = pers.tile([128, 2, M2], bf16, name="out2T")
            for nb in range(nblocks):
                xcol = rotp.tile([128, 9, NB], bf16, tag="xcol")
                nc.sync.dma_start(xcol[:], xT[:, :, nb * NB:(nb + 1) * NB])
                mcol = rotp.tile([128, 2, NB], bf16, tag="mcol")
                nc.sync.dma_start(mcol[:], m1T[:, :, nb * NB:(nb + 1) * NB])
                o1c = rotp.tile([128, 2, NB], bf16, tag="o1c")
                for m in range(2):
                    ps1 = psB.tile([128, 512], f32, tag="ps512")
                    for kt in range(9):
                        nc.tensor.matmul(ps1[:, 0:NB], w0_s[:, kt, 128 * m:128 * (m + 1)],
                                         xcol[:, kt, :], start=(kt == 0), stop=(kt == 8))
                    nc.scalar.activation(o1c[:, m, :], ps1[:, 0:NB], AF.Relu)
                    nc.vector.tensor_tensor(o1c[:, m, :], o1c[:, m, :], mcol[:, m, :], ALU.mult)
                mcol2 = rotp.tile([128, 2, NB], bf16, tag="mcol2")
                nc.sync.dma_start(mcol2[:], m2T[:, :, nb * NB:(nb + 1) * NB])
                for m in range(2):
                    ps1 = psB.tile([128, 512], f32, tag="ps512")
                    for kt in range(2):
                        nc.tensor.matmul(ps1[:, 0:NB], w1_s[:, kt, 128 * m:128 * (m + 1)],
                                         o1c[:, kt, :], start=(kt == 0), stop=(kt == 1))
                    sl = (slice(None), m, slice(nb * NB, (nb + 1) * NB))
                    nc.scalar.activation(out2T[sl], ps1[:, 0:NB], AF.Relu)
                    nc.vector.tensor_tensor(out2T[sl], out2T[sl], mcol2[:, m, :], ALU.mult)
            nch = (M2 + 127) // 128
            for ch in range(nch):
                r0 = ch * 128
                rows = min(128, M2 - r0)
                ps1 = psB.tile([128, 512], f32, tag="ps512")
                nc.tensor.matmul(ps1[0:rows, :], out2T[:, 0, r0:r0 + rows], wpx_s[:, 0, :],
                                 start=True, stop=False)
                nc.tensor.matmul(ps1[0:rows, :], out2T[:, 1, r0:r0 + rows], wpx_s[:, 1, :],
                                 start=False, stop=False)
                nc.tensor.matmul(ps1[0:rows, :], ones_s[:, 0:rows], prb_s[:],
                                 start=False, stop=True)
                kb0 = rot.tile([128, GSH], bf16, tag="kb0")
                nc.scalar.activation(kb0[0:rows, :], ps1[0:rows, :], AF.Copy)
                nc.sync.dma_start(known_d[r0:r0 + rows, :], kb0[0:rows, :])
            pm = pers.tile([128, 512], bf16, name="procmem")
            ps1 = psB.tile([128, 512], f32, tag="ps512")
            for kt in range(4):
                nc.tensor.matmul(ps1[:], wmem_s[:, kt, :], encd_s[:, kt, :],
                                 start=(kt == 0), stop=(kt == 3))
            nc.vector.tensor_copy(pm[:], ps1[:])

            # ================= LOOP =================
            curA_prev = curA0
            ctxT_prev = ctxT0
            for t in range(T_loop):
                kb = known_buf[t % 3]
                nc.sync.dma_start(kb[0:16, :], known_d[16 * t:16 * t + 16, :])
                hm = rot.tile([16, 256], f32, tag="hm")
                nc.sync.dma_start(hm[:], hmask[t])

                # ---- pre-gates
                gp = psA.tile([16, GSH], f32, tag="gates")
                nc.tensor.matmul(gp[:], id_s[:, 0:16], kb[:], start=True, stop=False)
                for r in range(8):
                    nc.tensor.matmul(gp[:], curA_prev[:, r, 0:16], whp_s[:, r, :],
                                     start=False, stop=False)
                for ko in range(4):
                    nc.tensor.matmul(gp[:], ctxT_prev[:, ko, :], wpc_s[:, ko, :],
                                     start=False, stop=(ko == 3))

                sif = rot.tile([16, 256], f32, tag="sif_a")
                nc.scalar.activation(sif[:], gp[:, 0:256], AF.Sigmoid)
                tg = rot.tile([16, 128], f32, tag="tg_a")
                nc.scalar.activation(tg[:], gp[:, 256:384], AF.Tanh)
                so = rot.tile([16, 128], f32, tag="so_a")
                nc.scalar.activation(so[:], gp[:, 384:512], AF.Sigmoid)
                t1 = rot.tile([16, 128], f32, tag="t1_a")
                nc.vector.tensor_tensor(t1[:], sif[:, 0:128], tg[:], ALU.mult)
                nc.vector.tensor_tensor(pre_c[:], sif[:, 128:256], pre_c[:], ALU.mult)
                nc.vector.tensor_tensor(pre_c[:], pre_c[:], t1[:], ALU.add)
                tch_ = rot.tile([16, 128], f32, tag="tc_a")
                nc.scalar.activation(tch_[:], pre_c[:], AF.Tanh)
                h2 = rot.tile([16, 128], f32, tag="h2_a")
                nc.vector.tensor_tensor(h2[:], so[:], tch_[:], ALU.mult)
                h2b = rot.tile([16, 128], bf16, tag="h2b_a")
                nc.vector.tensor_tensor(h2b[:], h2[:], hm[:, 0:128], ALU.mult)
                eA = exchA_src[t % 2]
                tp = psT.tile([128, 16], bf16, tag="tp")
                nc.tensor.transpose(tp[:], h2b[:], id_s[0:16, 0:16])
                nc.vector.tensor_copy(eA[:, 0:16], tp[:])

                # ---- AllGather A
                binA = dram.tile([128, 32], bf16, tag="binA")
                nc.gpsimd.dma_start(binA[:], eA[:])
                for _du in range(int(os.environ.get("KDUPAG", "1"))):
                    nc.gpsimd.collective_compute(
                        "AllGather", ALU.bypass, replica_groups=RG,
                        ins=[binA[:].opt()], outs=[histA_d[t].opt()])
                curA = rot.tile([128, 8, 32], bf16, tag="curA")
                nc.sync.dma_start(
                    curA[:], bass.AP(histA_d.tensor, histA_d.offset + t * 32768,
                                     [[32, 128], [4096, 8], [1, 32]]))

                # ---- conv (uses align/cum state from step t-1)
                im = imc[t % 2]
                lr = locrep_d[t % 2]
                for ci, tiles_ in ((0, align_bf), (1, cum_bf)):
                    for b in range(2):
                        dst = bass.AP(lr.tensor, lr.offset + ci * 17732 + b * 286,
                                      [[35464, 1], [572, 31], [1, 286]])
                        nc.sync.dma_start(dst, tiles_[b][:].unsqueeze(1).to_broadcast((1, 31, 286)))
                for ci in range(2):
                    src_ap = bass.AP(lr.tensor, lr.offset + ci * 17732,
                                     [[573, 31], [286, 2], [1, 256]])
                    nc.sync.dma_start(
                        im[31 * ci:31 * (ci + 1), :].rearrange("p (b t) -> p b t", b=2), src_ap)
                cps = psB.tile([128, 512], f32, tag="ps512")
                nc.tensor.matmul(cps[:], wf_s[:], im[:], start=True, stop=False)
                nc.tensor.matmul(cps[:], id_s[:], pm[:], start=False, stop=True)

                # ---- q^T for all 16 rows, select ours via partition id
                qps = psT2.tile([128, 16], f32, tag="qp")
                for r in range(8):
                    nc.tensor.matmul(qps[:, 0:16], wq_s[:, r, :], curA[:, r, 0:16],
                                     start=(r == 0), stop=(r == 7))
                q_sb = rot.tile([128, 2], f32, tag="q_sb")
                nc.vector.tensor_copy(q_sb[:], qps[:, ds(boff, 2)])

                # ---- tanh(s + q_b)
                th = rot.tile([128, 512], bf16, tag="tanh_sb")
                for b in range(2):
                    nc.scalar.activation(th[:, 256 * b:256 * (b + 1)], cps[:, 256 * b:256 * (b + 1)],
                                         AF.Tanh, bias=q_sb[:, b:b + 1], scale=1.0)

                # ---- e = v.tanh -> mask -> exp (+sums)
                eps_ = psB.tile([128, 512], f32, tag="ps512")
                nc.tensor.matmul(eps_[0:1, :], v_s[:], th[:], start=True, stop=False)
                nc.tensor.matmul(eps_[0:1, :], ones_s[:, 0:1], lm_s[:], start=False, stop=True)
                sp = sums_pad[t % 2]
                nc.any.memset(sp[0:1, :], 0.0)
                exp_sb = rot.tile([1, 2, ENC_T], f32, tag="exp_sb")
                for b in range(2):
                    nc.scalar.activation(exp_sb[0:1, b, :], eps_[0:1, 256 * b:256 * (b + 1)],
                                         AF.Exp, accum_out=sp[0:1, b:b + 1])
                rps = psT2.tile([128, 2], f32, tag="qp")
                nc.tensor.matmul(rps[:, 0:2], onesf_s[:], sp[:], start=True, stop=True)
                recip = rot.tile([128, 2], f32, tag="recip")
                nc.vector.reciprocal(recip[:], rps[:, 0:2])

                # ---- e^T path for ctx
                etps = psB.tile([128, 512], f32, tag="ps512")
                for b in range(2):
                    for tt in range(2):
                        j = 2 * b + tt
                        nc.tensor.matmul(etps[:, j:j + 1],
                                         th[:, 256 * b + 128 * tt:256 * b + 128 * (tt + 1)],
                                         v_s[:], start=True, stop=False)
                eT_f = rot.tile([128, 4], f32, tag="eT_f")
                nc.vector.tensor_tensor(eT_f[:], etps[:, 0:4], lmT_s[:], ALU.add)
                expT = rot.tile([128, 4], bf16, tag="expT")
                nc.scalar.activation(expT[:], eT_f[:], AF.Exp)

                # ---- align, cum, bf16 copies
                align_f = rot.tile([1, 2, ENC_T], f32, tag="align_f")
                for b in range(2):
                    nc.vector.tensor_tensor(align_f[0:1, b, :], exp_sb[0:1, b, :],
                                            recip[0:1, b:b + 1].to_broadcast((1, ENC_T)),
                                            ALU.mult)
                nc.sync.dma_start(out_align[t], align_f[:])
                nc.vector.tensor_tensor(cum_f[:], cum_f[:], align_f[:], ALU.add)
                for b in range(2):
                    nc.vector.tensor_copy(align_bf[b][:, 15:271], align_f[0:1, b, :])
                    nc.vector.tensor_copy(cum_bf[b][:, 15:271], cum_f[0:1, b, :])

                # ---- ctx^T direct (scaled by recip)
                eB = exchB_src[t % 2]
                for b in range(2):
                    xps = psB.tile([128, 512], f32, tag="ps512")
                    for ko in range(4):
                        for tt in range(2):
                            nc.tensor.matmul(xps[:, ko:ko + 1],
                                             mem_s[:, 2 * b + tt, 128 * ko:128 * (ko + 1)],
                                             expT[:, 2 * b + tt:2 * b + tt + 1],
                                             start=(tt == 0), stop=(tt == 1))
                    nc.scalar.activation(eB[:, 4 * b:4 * (b + 1)], xps[:, 0:4], AF.Copy,
                                         bias=0.0, scale=recip[:, b:b + 1])

                # ---- AllGather B
                binB = dram.tile([128, 8], bf16, tag="binB")
                nc.gpsimd.dma_start(binB[:], eB[:])
                for _du in range(int(os.environ.get("KDUPAG", "1"))):
                    nc.gpsimd.collective_compute(
                        "AllGather", ALU.bypass, replica_groups=RG,
                        ins=[binB[:].opt()], outs=[histC_d[t + 1].opt()])
                gB = rot.tile([128, 8, 8], bf16, tag="gB")
                nc.sync.dma_start(
                    gB[:], bass.AP(histC_d.tensor, histC_d.offset + (t + 1) * 8192,
                                   [[8, 128], [1024, 8], [1, 8]]))
                ctxT = rot.tile([128, 4, 16], bf16, tag="ctxT")
                # shuffle [p, r, (bb,ko)] -> [p, ko, (r,bb)]
                nc.vector.tensor_copy(
                    bass.AP(ctxT.tensor, ctxT.offset, [[64, 128], [16, 4], [2, 8], [1, 2]]),
                    bass.AP(gB.tensor, gB.offset, [[64, 128], [1, 4], [8, 8], [4, 2]]))

                # ---- post-gates
                gq = psA.tile([16, GSH], f32, tag="gates")
                nc.tensor.matmul(gq[:], ones_s[:, 0:16], pob_s[:], start=True, stop=False)
                for r in range(8):
                    nc.tensor.matmul(gq[:], curA[:, r, 0:16], wph_s[:, r, :],
                                     start=False, stop=False)
                for r in range(8):
                    nc.tensor.matmul(gq[:], curA[:, r, 16:32], whq_s[:, r, :],
                                     start=False, stop=False)
                for ko in range(4):
                    nc.tensor.matmul(gq[:], ctxT[:, ko, :], wpc2_s[:, ko, :],
                                     start=False, stop=(ko == 3))

                sif2 = rot.tile([16, 256], f32, tag="sif_b")
                nc.scalar.activation(sif2[:], gq[:, 0:256], AF.Sigmoid)
                tg2 = rot.tile([16, 128], f32, tag="tg_b")
                nc.scalar.activation(tg2[:], gq[:, 256:384], AF.Tanh)
                so2 = rot.tile([16, 128], f32, tag="so_b")
                nc.scalar.activation(so2[:], gq[:, 384:512], AF.Sigmoid)
                t12 = rot.tile([16, 128], f32, tag="t1_b")
                nc.vector.tensor_tensor(t12[:], sif2[:, 0:128], tg2[:], ALU.mult)
                nc.vector.tensor_tensor(post_c[:], sif2[:, 128:256], post_c[:], ALU.mult)
                nc.vector.tensor_tensor(post_c[:], post_c[:], t12[:], ALU.add)
                tc22 = rot.tile([16, 128], f32, tag="tc_b")
                nc.scalar.activation(tc22[:], post_c[:], AF.Tanh)
                h22 = rot.tile([16, 128], f32, tag="h2_b")
                nc.vector.tensor_tensor(h22[:], so2[:], tc22[:], ALU.mult)
                h22b = rot.tile([16, 128], bf16, tag="h2b_b")
                nc.vector.tensor_tensor(h22b[:], h22[:], hm[:, 128:256], ALU.mult)
                eA2 = exchA_src[(t + 1) % 2]
                tp2 = psT.tile([128, 16], bf16, tag="tp")
                nc.tensor.transpose(tp2[:], h22b[:], id_s[0:16, 0:16])
                nc.vector.tensor_copy(eA2[:, 16:32], tp2[:])

                curA_prev = curA
                ctxT_prev = ctxT

            # final exchange (post_h_{T-1})
            binA = dram.tile([128, 32], bf16, tag="binA")
            nc.gpsimd.dma_start(binA[:], exchA_src[T_loop % 2][:])
            nc.gpsimd.collective_compute(
                "AllGather", ALU.bypass, replica_groups=RG,
                ins=[binA[:].opt()], outs=[histA_d[T_loop].opt()])

            # ================= SUFFIX: projection =================
            for ch in range(nch):
                r0 = ch * 128
                rows = min(128, M2 - r0)
                tch = r0 // 16
                ts_n = rows // 16
                lb = rotp.tile([128, 8, 128], bf16, tag="lb")
                for r in range(8):
                    nc.sync.dma_start(
                        bass.AP(lb.tensor, lb.offset + r * 128,
                                [[1024, 128], [16, ts_n], [1, 16]]),
                        bass.AP(histA_d.tensor,
                                histA_d.offset + (tch + 1) * 32768 + r * 4096 + 16,
                                [[32, 128], [32768, ts_n], [1, 16]]))
                lbc = rotp.tile([128, 4, 8, 16], bf16, tag="lbc")
                for tq in range(ts_n):
                    gBs = rotp.tile([128, 8, 8], bf16, tag="gBs")
                    nc.sync.dma_start(
                        gBs[:], bass.AP(histC_d.tensor,
                                        histC_d.offset + (tch + 1 + tq) * 8192,
                                        [[8, 128], [1024, 8], [1, 8]]))
                    nc.vector.tensor_copy(
                        bass.AP(lbc.tensor, lbc.offset + tq * 16,
                                [[512, 128], [128, 4], [2, 8], [1, 2]]),
                        bass.AP(gBs.tensor, gBs.offset, [[64, 128], [1, 4], [8, 8], [4, 2]]))
                pp = psB.tile([128, 512], f32, tag="ps512")
                for kt in range(8):
                    nc.tensor.matmul(pp[0:rows, 0:PROJ_W], lb[:, kt, 0:rows],
                                     wproj_s[:, kt, :], start=(kt == 0), stop=False)
                for ko in range(4):
                    lhs = bass.AP(lbc.tensor, lbc.offset + ko * 128,
                                  [[512, 128], [1, rows]])
                    nc.tensor.matmul(pp[0:rows, 0:PROJ_W], lhs,
                                     wproj_s[:, 8 + ko, :], start=False, stop=False)
                nc.tensor.matmul(pp[0:rows, 0:PROJ_W], ones_s[:, 0:rows], pjb_s[:],
                                 start=False, stop=True)
                ob = rotp.tile([128, PROJ_W], f32, tag="ob")
                nc.scalar.activation(ob[0:rows, :], pp[0:rows, 0:PROJ_W], AF.Copy)
                nc.sync.dma_start(out_proj[r0:r0 + rows, :], ob[0:rows, :])

    nc.compile()
    return nc


def _make_masks(n_steps):
    """Bit-exact reproduction of the reference's threefry dropout masks."""
    import jax
    import jax.numpy as jnp
    cpu = jax.devices("cpu")[0]
    with jax.default_device(cpu):
        dk = jax.random.key(42)

        def dmask(key, shape, p):
            keep = jax.random.bernoulli(key, 1.0 - p, shape)
            return np.asarray(jnp.where(keep, 1.0 / (1.0 - p), 0.0), np.float32)

        @jax.jit
        def hmasks(t):
            kp = jax.random.fold_in(dk, 2 + 2 * t)
            kq = jax.random.fold_in(dk, 3 + 2 * t)
            return (jnp.where(jax.random.bernoulli(kp, 0.9, (B, H)), 1.0 / 0.9, 0.0),
                    jnp.where(jax.random.bernoulli(kq, 0.9, (B, H)), 1.0 / 0.9, 0.0))

        m1 = dmask(jax.random.fold_in(dk, 0), (B, n_steps, PRENET), 0.5)
        m2 = dmask(jax.random.fold_in(dk, 1), (B, n_steps, PRENET), 0.5)
        mp = np.zeros((n_steps, B, H), np.float32)
        mq = np.zeros((n_steps, B, H), np.float32)
        for t in range(n_steps):
            a, b2 = hmasks(t)
            mp[t] = np.asarray(a, np.float32)
            mq[t] = np.asarray(b2, np.float32)
    return m1, m2, mp, mq


def kernel(encodings, encoding_lengths, features, go_frame, prenet_w0, prenet_w1,
           pre_wih, pre_whh, pre_b, attn_query_w, attn_memory_w, attn_loc_conv,
           attn_loc_w, attn_v, post_wih, post_whh, post_b, proj_w, proj_b):
    n_steps = int(features.shape[2])
    T = n_steps
    M2 = 16 * T

    # ---------- host-side marshalling ----------
    feats = np.concatenate(
        [np.broadcast_to(np.asarray(go_frame, np.float32), (B, FEAT, 1)),
         np.asarray(features, np.float32)[:, :, :-1]], axis=2)   # [B, FEAT, T]
    x = feats.transpose(2, 0, 1).reshape(M2, FEAT)               # rows (t, b)
    xT_np = np.zeros((XK, M2), np.float32)
    xT_np[:FEAT, :] = x.T
    m1, m2, mp, mq = _make_masks(T)
    m1T = m1.transpose(2, 1, 0).reshape(PRENET, T * B)           # wrong order fix below
    # masks are [B, T, 256]; we need [256, (t,b)] with b inner:
    m1T = np.ascontiguousarray(np.transpose(m1, (2, 1, 0)).reshape(PRENET, M2))
    m2T = np.ascontiguousarray(np.transpose(m2, (2, 1, 0)).reshape(PRENET, M2))

    w0p = np.zeros((XK, PRENET), np.float32)
    w0p[:FEAT] = np.asarray(prenet_w0, np.float32)
    w1_np = np.asarray(prenet_w1, np.float32)

    pre_wih = np.asarray(pre_wih, np.float32)
    pre_whh = np.asarray(pre_whh, np.float32)
    pre_b = np.asarray(pre_b, np.float32)
    post_wih = np.asarray(post_wih, np.float32)
    post_whh = np.asarray(post_whh, np.float32)
    post_b = np.asarray(post_b, np.float32)
    proj_w_np = np.asarray(proj_w, np.float32)
    proj_b_np = np.asarray(proj_b, np.float32)
    enc = np.asarray(encodings, np.float32)
    lens = np.asarray(encoding_lengths).astype(np.int64)

    # fused location conv+proj: Wf[(c,k), a] = sum_o conv[o,c,k] * loc_w[o,a]
    conv = np.asarray(attn_loc_conv, np.float32)       # [32, 2, 31]
    locw = np.asarray(attn_loc_w, np.float32)          # [32, 128]
    wf_np = np.einsum('ock,oa->cka', conv, locw).reshape(62, ATTN)
    wf_pad = np.zeros((128, ATTN), np.float32)
    wf_pad[:62] = wf_np

    v_np = np.asarray(attn_v, np.float32).reshape(ATTN, 1)
    wq_np = np.asarray(attn_query_w, np.float32)       # [1024, 128]
    wmem_np = np.asarray(attn_memory_w, np.float32)    # [512, 128]

    ident = np.eye(128, dtype=np.float32)
    ones_row = np.zeros((128, 128), np.float32)
    ones_row[0, :] = 1.0

    ck = (T, os.environ.get("KLOOPT", ""), os.environ.get("KDUPAG", ""))
    if ck not in _nc_cache:
        _nc_cache[ck] = build_nc(T)
    nc = _nc_cache[ck]

    in_maps = []
    for c in range(NC):
        cols = np.concatenate([np.arange(128 * c, 128 * (c + 1)) + H * k for k in range(4)])
        hsl = slice(128 * c, 128 * (c + 1))
        bsl = slice(2 * c, 2 * c + 2)

        # attention per-core data
        mem_c = enc[bsl]                               # [2, 512, 256]
        # mem_t [p, (b,tt), d]: element = enc[2c+b, d, tt*128+p]
        mt = mem_c.transpose(2, 0, 1).reshape(2, 128, 2, 512, order='F') if False else None
        mt = np.zeros((128, 4, ENC_D), np.float32)
        for b in range(2):
            for tt in range(2):
                mt[:, 2 * b + tt, :] = mem_c[b, :, 128 * tt:128 * (tt + 1)].T
        # enc_dT [p, ko, (b,t)]: element = enc[2c+b, ko*128+p, t]
        ed = np.zeros((128, 4, 512), np.float32)
        for ko in range(4):
            for b in range(2):
                ed[:, ko, 256 * b:256 * (b + 1)] = mem_c[b, 128 * ko:128 * (ko + 1), :]
        lmask = np.zeros((2, ENC_T), np.float32)
        for b in range(2):
            lmask[b, lens[2 * c + b]:] = -1e9
        lmask_mm = np.zeros((128, 512), np.float32)
        lmask_mm[0, :] = lmask.reshape(512)
        lmaskT = np.zeros((128, 4), np.float32)
        for b in range(2):
            for tt in range(2):
                lmaskT[:, 2 * b + tt] = lmask[b, 128 * tt:128 * (tt + 1)]

        hm = np.concatenate([mp[:, :, hsl], mq[:, :, hsl]], axis=2)  # [T, 16, 256]

        pw = np.zeros((1536, PROJ_W), np.float32)
        pcols = slice(128 * c, 128 * c + PROJ_W)
        pw[:, :] = proj_w_np[:, pcols]
        pjb = np.zeros((128, PROJ_W), np.float32)
        pjb[0, :] = proj_b_np[pcols]

        prb = np.zeros((128, GSH), np.float32)
        prb[0, :] = pre_b[cols]
        pob = np.zeros((128, GSH), np.float32)
        pob[0, :] = post_b[cols]

        in_maps.append({
            "xT": _bf(_ktile(xT_np)),
            "w0": _bf(_ktile(w0p)),
            "w1": _bf(_ktile(w1_np)),
            "m1T": _bf(_ktile(m1T)),
            "m2T": _bf(_ktile(m2T)),
            "wih_pre_x": _bf(_ktile(pre_wih[0:256][:, cols])),
            "preb_row": _bf(prb),
            "wih_pre_ctx": _bf(_ktile(pre_wih[256:768][:, cols])),
            "whh_pre": _bf(_ktile(pre_whh[:, cols])),
            "wih_post_h": _bf(_ktile(post_wih[0:1024][:, cols])),
            "wih_post_ctx": _bf(_ktile(post_wih[1024:1536][:, cols])),
            "whh_post": _bf(_ktile(post_whh[:, cols])),
            "postb_row": _bf(pob),
            "wq": _bf(_ktile(wq_np)),
            "wmem": _bf(_ktile(wmem_np)),
            "wf": _bf(wf_pad),
            "vattn": _bf(v_np),
            "mem_t": _bf(mt),
            "enc_dT": _bf(ed),
            "lenmask": _bf(lmask_mm),
            "lenmaskT": _bf(lmaskT),
            "hmask": np.ascontiguousarray(hm),
            "ident": _bf(ident),
            "ones_row": _bf(ones_row),
            "ones_f32": np.ascontiguousarray(ones_row),
            "wproj": _bf(_ktile(pw)),
            "projb_row": _bf(pjb),
        })

    import time as _time
    _t0 = _time.time()
    res = run_bass_kernel_spmd(nc, in_maps, core_ids=list(range(NC)),
                               trace=bool(int(os.environ.get("KTRACE", "0"))))
    kernel.last_run_s = _time.time() - _t0
    kernel.last_results = res

    # ---------- output assembly ----------
    proj_full = np.zeros((M2, FEAT), np.float32)
    for c in range(NC):
        op = res.results[c]["out_proj"]                # [M2, 129]
        if c < NC - 1:
            proj_full[:, 128 * c:128 * (c + 1)] = op[:, 0:128]
        else:
            proj_full[:, 128 * c:128 * c + PROJ_W] = op
    projs = proj_full.reshape(T, B, FEAT).transpose(1, 2, 0)      # [B, FEAT, T]

    aligns = np.zeros((B, ENC_T, T), np.float32)
    for c in range(NC):
        oa = res.results[c]["out_align"]               # [T, 2, 256]
        aligns[2 * c:2 * c + 2] = oa.transpose(1, 2, 0)
    return projs, aligns
